# revision 1
# baseline (speedup 1.0000x reference)
"""Grouped SwiGLU MoE expert FFN on 8 Trainium2 NeuronCores.

Problem: out[t] = w2[e(t)] @ (silu(w1[e(t)] x[t]) * (w3[e(t)] x[t])),
T=4096 tokens sorted by expert, E=8 experts, H=1024, I=2816, fp32.

Strategy (expert-parallel + token balancing, no collectives):
  - Token groups (per expert, multiples of 128) are decomposed into eight
    384-token pieces and eight 128-token pieces; each core gets exactly one
    of each (512 tokens), so compute is perfectly balanced across cores.
  - Each core's SPMD program has two weight slots (the two pieces'
    experts). Weights are streamed from HBM once per slot.
  - Matmul layout keeps weights stationary and tokens moving:
      u^T[i] = sum_k w1[k,i]^T x^T[k,:]   (PSUM, accumulate over H tiles)
      h^T[i] = silu(u^T[i]) * v^T[i]      (ACT + DVE)
      out^T[m] = sum_i w2[i,m]^T h^T[i]   (PSUM, accumulate over I tiles)
    x^T / weight blocks are pre-laid-out on the host so every DMA is
    fully contiguous per partition.
  - dtype variants (MOE_VARIANT env, default fp16):
      "fp16": weights/activations cast to fp16 (values are O(1), fp16-safe);
              ~129 us/core model time, ~5e-4 rel-L2 vs the fp32 reference.
      "f32r": fp32 bits at full PE rate via float32r; exactness-oriented
              (~2.6e-4), DMA-bound ~230 us. f32r needs >=256 moving rows, so
              the 128-piece is computed as rows [256:512] overlapping the
              384-piece and the duplicated rows are discarded on the host.
      "bf16": like fp16 but bf16 (~4e-3) — dominated by fp16, kept for tests.

Self-contained: only needs numpy + the concourse/axon runtime.
"""

import numpy as np
import ml_dtypes

import jax
import concourse.tile as tile
from concourse import bacc, mybir

E, H, II = 8, 1024, 2816
NCORES = 8
TOK = 512            # tokens per core
KT = H // 128        # 8  contraction tiles (stage 1)
IT = II // 128       # 22 intermediate tiles
MT = H // 128        # 8  output tiles

F32 = mybir.dt.float32
F32R = mybir.dt.float32r
BF16 = mybir.dt.bfloat16
FP16 = mybir.dt.float16

import os
VARIANT = os.environ.get("MOE_VARIANT", "fp16")  # "f32r" | "bf16"


# ----------------------------------------------------------------------------
# Planning: decompose ragged groups into eight 384-pieces + eight 128-pieces.
# ----------------------------------------------------------------------------

def _plan_pieces(group_sizes):
    """Return (pieces384, pieces128): lists of (expert, tok_start), 8 each.

    Requires all group sizes to be multiples of 128 and sum to 4096.
    """
    g = [int(v) for v in group_sizes]
    if sum(g) != NCORES * TOK or any(v % 128 for v in g) or len(g) != E:
        return None
    offs = np.concatenate([[0], np.cumsum(g)])
    # x_e = number of 384-pieces for expert e; need sum(x)=8, 384*x_e <= g_e.
    x = [v // 384 for v in g]
    total = sum(x)
    if total < 8:
        return None
    e_i = 0
    while total > 8:  # shed surplus 384-pieces (each becomes three 128s)
        if x[e_i] > 0:
            x[e_i] -= 1
            total -= 1
        e_i = (e_i + 1) % E
    p384, p128 = [], []
    for e in range(E):
        t = int(offs[e])
        for _ in range(x[e]):
            p384.append((e, t))
            t += 384
        while t < offs[e + 1]:
            p128.append((e, t))
            t += 128
    if len(p384) != 8 or len(p128) != 8:
        return None
    return p384, p128


# ----------------------------------------------------------------------------
# Device program
# ----------------------------------------------------------------------------

def _build_program(chunks, wdt, hdt, out_cols):
    """chunks: list of (row0, nrows, slot). out_cols[j] = cols of output j.

    Schedule: both chunks' phase-1 first (interleaved weight streaming),
    then both phase-2s; buffer counts tuned via TimelineSim.
    """
    nc = bacc.Bacc()
    nslots = 1 + max(s for _, _, s in chunks)
    nchunks = len(chunks)

    xTs = [
        nc.declare_dram_parameter(f"xT_{j}", [128, KT, nr], wdt, isOutput=False)
        for j, (_, nr, _) in enumerate(chunks)
    ]
    wp = {}
    for s in range(nslots):
        wp[s, "w1"] = nc.declare_dram_parameter(f"w1_{s}", [IT, 128, KT * 128], wdt, isOutput=False)
        wp[s, "w3"] = nc.declare_dram_parameter(f"w3_{s}", [IT, 128, KT * 128], wdt, isOutput=False)
        wp[s, "w2"] = nc.declare_dram_parameter(f"w2_{s}", [IT, 128, MT * 128], wdt, isOutput=False)
    outs = [
        nc.declare_dram_parameter(f"outT_{j}", [MT, 128, c], F32, isOutput=True)
        for j, c in enumerate(out_cols)
    ]

    is_fp32 = wdt == F32R
    # fp32 tiles are 2x the size; shrink pools to fit SBUF
    w13_bufs = 4 if is_fp32 else 6
    w2_bufs = nchunks * IT + 1 if not is_fp32 else IT + 1
    h_bufs = nchunks * IT + 2 if not is_fp32 else IT + 2
    interleave = not is_fp32

    with tile.TileContext(nc) as tc:
        with tc.tile_pool(name="xp", bufs=1) as xp, \
             tc.tile_pool(name="w13", bufs=w13_bufs) as w13p, \
             tc.tile_pool(name="w2", bufs=w2_bufs) as w2p, \
             tc.tile_pool(name="h", bufs=h_bufs) as hp, \
             tc.tile_pool(name="su", bufs=4) as sup, \
             tc.tile_pool(name="oc", bufs=6) as ocp, \
             tc.tile_pool(name="up", bufs=3, space="PSUM") as up, \
             tc.tile_pool(name="vp", bufs=3, space="PSUM") as vp, \
             tc.tile_pool(name="op", bufs=2, space="PSUM") as op:

            warm = sup.tile([1, 16], wdt, tag="warm")
            nc.vector.memset(warm[:], 0.0)
            wps = op.tile([16, 16], F32, tag="o", name="warmps")
            for _ in range(64):
                nc.tensor.matmul(wps[:], warm[:], warm[:], start=True, stop=True)

            xts = []
            for j, (_, nr, _) in enumerate(chunks):
                xt_j = xp.tile([128, KT, nr], wdt, name=f"xt{j}", tag=f"xt{j}")
                nc.sync.dma_start(out=xt_j[:], in_=xTs[j][:])
                xts.append(xt_j)

            w2ts = {}
            hts = {}

            def phase1(ci):
                r0, nr, s = chunks[ci]
                w2ts[ci] = []
                hts[ci] = []
                for i in range(IT):
                    w1t = w13p.tile([128, KT * 128], wdt, tag="w1")
                    nc.sync.dma_start(out=w1t[:], in_=wp[s, "w1"][i])
                    w3t = w13p.tile([128, KT * 128], wdt, tag="w3")
                    nc.sync.dma_start(out=w3t[:], in_=wp[s, "w3"][i])
                    w2t = w2p.tile([128, MT * 128], wdt, tag="w2")
                    nc.sync.dma_start(out=w2t[:], in_=wp[s, "w2"][i])
                    w2ts[ci].append(w2t)

                    u = up.tile([128, nr], F32, tag="u")
                    v = vp.tile([128, nr], F32, tag="v")
                    w1r = w1t[:].rearrange("p (k j) -> p k j", k=KT)
                    w3r = w3t[:].rearrange("p (k j) -> p k j", k=KT)
                    for k in range(KT):
                        nc.tensor.matmul(
                            u[:], w1r[:, k, :], xts[ci][:, k, :],
                            start=(k == 0), stop=(k == KT - 1),
                        )
                    for k in range(KT):
                        nc.tensor.matmul(
                            v[:], w3r[:, k, :], xts[ci][:, k, :],
                            start=(k == 0), stop=(k == KT - 1),
                        )
                    su = sup.tile([128, nr], F32, tag="su")
                    nc.scalar.activation(
                        out=su[:], in_=u[:],
                        func=mybir.ActivationFunctionType.Silu,
                    )
                    ht = hp.tile([128, nr], hdt, tag="h")
                    nc.vector.tensor_mul(ht[:], su[:], v[:])
                    hts[ci].append(ht)

            def phase2(ci):
                r0, nr, s = chunks[ci]
                for m in range(MT):
                    o = op.tile([128, nr], F32, tag="o")
                    for i in range(IT):
                        w2r = w2ts[ci][i][:].rearrange("p (m j) -> p m j", m=MT)
                        nc.tensor.matmul(
                            o[:], w2r[:, m, :], hts[ci][i][:],
                            start=(i == 0), stop=(i == IT - 1),
                        )
                    oc = ocp.tile([128, nr], F32, tag="oc")
                    nc.vector.tensor_copy(out=oc[:], in_=o[:])
                    nc.sync.dma_start(out=outs[ci][m], in_=oc[:])

            if interleave:
                for ci in range(nchunks):
                    phase1(ci)
                for ci in range(nchunks):
                    phase2(ci)
            else:
                for ci in range(nchunks):
                    phase1(ci)
                    phase2(ci)

    nc.finalize()
    return nc


# ----------------------------------------------------------------------------
# Host-side data prep
# ----------------------------------------------------------------------------

def _np_dtype(variant):
    if variant == "f32r":
        return np.float32
    return np.float16 if variant == "fp16" else ml_dtypes.bfloat16


def _fmt_w13(w, dt):
    # [H, I] -> [IT, 128, KT*128]; block i, partition p, col k*128+j = w[k*128+p, i*128+j]
    return np.ascontiguousarray(
        w.reshape(KT, 128, IT, 128).transpose(2, 1, 0, 3).reshape(IT, 128, KT * 128)
    ).astype(dt)


def _fmt_w2(w, dt):
    # [I, H] -> [IT, 128, MT*128] (already contiguous blocks)
    return np.ascontiguousarray(w.reshape(IT, 128, MT * 128)).astype(dt)


def _fmt_xT(x, dt):
    # [n, H] -> [128, KT, n]; partition p, k, t = x[t, k*128+p]
    n = x.shape[0]
    return np.ascontiguousarray(x.T.reshape(KT, 128, n).transpose(1, 0, 2)).astype(dt)


_CACHE = {}


def _get_runner(variant, gs_key, group_sizes):
    """Build (or fetch) the compiled SPMD runner for these group sizes."""
    key = (variant, gs_key)
    if key in _CACHE:
        return _CACHE[key]

    plan = _plan_pieces(group_sizes)
    if plan is None:
        raise NotImplementedError(
            f"group_sizes {list(group_sizes)} not decomposable into 384/128 pieces"
        )
    p384, p128 = plan

    if variant == "f32r":
        wdt = hdt = F32R
        chunks = [(0, 384, 0), (256, 256, 1)]
        out_cols = [384, 256]
        bcol0 = 128  # cols of chunk-1 output corresponding to the 128-piece
    else:
        wdt = hdt = FP16 if variant == "fp16" else BF16
        chunks = [(0, 384, 0), (384, 128, 1)]
        out_cols = [384, 128]
        bcol0 = 0

    nc = _build_program(chunks, wdt, hdt, out_cols)
    runner = _make_pjrt_runner(nc)
    st = {
        "nc": nc, "runner": runner, "p384": p384, "p128": p128,
        "variant": variant, "bcol0": bcol0, "chunks": chunks,
    }
    _CACHE[key] = st
    return st


def _make_pjrt_runner(nc):
    """Persistent jit'd SPMD executor (mirrors bass2jax.run_bass_via_pjrt)."""
    from jax.sharding import Mesh, PartitionSpec
    from jax.experimental.shard_map import shard_map
    from concourse.bass2jax import (
        _bass_exec_p, install_neuronx_cc_hook, partition_id_tensor,
    )

    install_neuronx_cc_hook()

    partition_name = nc.partition_id_tensor.name if nc.partition_id_tensor else None
    in_names, out_names, out_avals = [], [], []
    for alloc in nc.m.functions[0].allocations:
        if not isinstance(alloc, mybir.MemoryLocationSet):
            continue
        name = alloc.memorylocations[0].name
        if alloc.kind == "ExternalInput":
            if name != partition_name:
                in_names.append(name)
        elif alloc.kind == "ExternalOutput":
            out_names.append(name)
            out_avals.append(
                jax.core.ShapedArray(tuple(alloc.tensor_shape), mybir.dt.np(alloc.dtype))
            )
    n_params = len(in_names)
    n_outs = len(out_names)
    all_in_names = list(in_names) + list(out_names)
    if partition_name is not None:
        all_in_names.append(partition_name)
    donate = tuple(range(n_params, n_params + n_outs))

    def _body(*args):
        operands = list(args)
        if partition_name is not None:
            operands.append(partition_id_tensor())
        outs = _bass_exec_p.bind(
            *operands,
            out_avals=tuple(out_avals),
            in_names=tuple(all_in_names),
            out_names=tuple(out_names),
            lowering_input_output_aliases=(),
            sim_require_finite=True,
            sim_require_nnan=True,
            nc=nc,
        )
        return tuple(outs)

    devices = jax.devices()[:NCORES]
    mesh = Mesh(np.asarray(devices), ("core",))
    in_specs = (PartitionSpec("core"),) * (n_params + n_outs)
    out_specs = (PartitionSpec("core"),) * n_outs
    jitted = jax.jit(
        shard_map(_body, mesh=mesh, in_specs=in_specs, out_specs=out_specs,
                  check_rep=False),
        donate_argnums=donate, keep_unused=True,
    )

    def run(in_maps):
        per_core = [[np.asarray(m[n]) for n in in_names] for m in in_maps]
        concat_in = [
            np.concatenate([per_core[c][i] for c in range(NCORES)], axis=0)
            for i in range(n_params)
        ]
        zeros = [
            np.zeros((NCORES * a.shape[0], *a.shape[1:]), a.dtype) for a in out_avals
        ]
        out_arrs = jitted(*concat_in, *zeros)
        return [
            {
                name: np.asarray(out_arrs[i]).reshape(NCORES, *out_avals[i].shape)[c]
                for i, name in enumerate(out_names)
            }
            for c in range(NCORES)
        ]

    return run


def _prep_in_maps(st, hidden_states, w1, w2, w3):
    dt = _np_dtype(st["variant"])
    w1f = [_fmt_w13(np.asarray(w1[e]), dt) for e in range(E)]
    w3f = [_fmt_w13(np.asarray(w3[e]), dt) for e in range(E)]
    w2f = [_fmt_w2(np.asarray(w2[e]), dt) for e in range(E)]
    hs = np.asarray(hidden_states)

    in_maps = []
    for c in range(NCORES):
        eA, tA = st["p384"][c]
        eB, tB = st["p128"][c]
        xc = np.concatenate([hs[tA:tA + 384], hs[tB:tB + 128]], axis=0)
        r1, n1 = st["chunks"][1][0], st["chunks"][1][1]
        in_maps.append({
            "xT_0": _fmt_xT(xc[0:384], dt),
            "xT_1": _fmt_xT(xc[r1:r1 + n1], dt),
            "w1_0": w1f[eA], "w3_0": w3f[eA], "w2_0": w2f[eA],
            "w1_1": w1f[eB], "w3_1": w3f[eB], "w2_1": w2f[eB],
        })
    return in_maps


def _assemble(st, results, out_dtype):
    out = np.empty((NCORES * TOK, H), dtype=out_dtype)
    bc = st["bcol0"]
    for c in range(NCORES):
        eA, tA = st["p384"][c]
        eB, tB = st["p128"][c]
        oA = results[c]["outT_0"].reshape(H, 384)   # [MT,128,384] -> [H,384]
        oB = results[c]["outT_1"]
        out[tA:tA + 384] = oA.T
        if oB.shape == (128, H):                    # token-major fast path
            out[tB:tB + 128] = oB
        else:
            oB = oB.reshape(H, oB.shape[-1])
            out[tB:tB + 128] = oB[:, bc:bc + 128].T
    return out


def kernel(hidden_states, group_sizes, w1, w2, w3):
    gs = np.asarray(group_sizes)
    st = _get_runner(VARIANT, gs.tobytes(), gs)
    in_maps = _prep_in_maps(st, hidden_states, w1, w2, w3)
    results = st["runner"](in_maps)
    return _assemble(st, results, np.asarray(hidden_states).dtype)



# revision 14
# speedup vs baseline: 1.0539x; 1.0539x over previous
"""Grouped SwiGLU MoE expert FFN on 8 Trainium2 NeuronCores.

Problem: out[t] = w2[e(t)] @ (silu(w1[e(t)] x[t]) * (w3[e(t)] x[t])),
T=4096 tokens sorted by expert, E=8 experts, H=1024, I=2816, fp32.

Strategy (expert-parallel + token balancing, no collectives):
  - Token groups (per expert, multiples of 128) are decomposed into eight
    384-token pieces and eight 128-token pieces; each core gets exactly one
    of each (512 tokens), so compute is perfectly balanced across cores.
  - Each core's SPMD program has two weight slots (the two pieces'
    experts). Weights are streamed from HBM once per slot, fp16
    (values are O(1), fp16-safe; ~5e-4 rel-L2 vs the fp32 reference).
  - Matmul layout keeps weights stationary and tokens moving:
      u^T[i] = sum_k w1[k,i]^T x^T[k,:]   (PSUM, accumulate over H tiles)
      h^T[i] = silu(u^T[i]) * v^T[i]      (ACT + DVE)
      out^T[m] = sum_i w2[i,m]^T h^T[i]   (PSUM, accumulate over I tiles)
  - Schedule (tuned against the TimelineSim cost model):
      * w1+w3 for an i-tile load as ONE merged DMA (fewer HWDGE/SEQ slots).
      * phase 1 interleaves the two chunks per i-tile so per-pair PE time
        (3.4us) exceeds per-pair DMA time (2.8us) everywhere - no bus
        crunch at the 128-chunk, and w2 tiles stream into the slack.
      * output stores issue from the Activation queue so they never convoy
        behind weight prefetches on SP.
      * a warmup matmul train keeps PE continuously busy until the first
        weights land, so the p-state ramp is complete (full 2.4 GHz) when
        the real stream starts and never resets.

Self-contained: only needs numpy + the concourse/axon runtime.
"""

import numpy as np

import jax
import concourse.tile as tile
from concourse import bacc, mybir

E, H, II = 8, 1024, 2816
NCORES = 8
TOK = 512            # tokens per core
KT = H // 128        # 8  contraction tiles (stage 1)
IT = II // 128       # 22 intermediate tiles
MT = H // 128        # 8  output tiles

F32 = mybir.dt.float32
FP16 = mybir.dt.float16

import os
VARIANT = os.environ.get("MOE_VARIANT", "fp16")

# warmup matmuls: keep PE busy (ramping) until the first weights arrive
WARMUP_N = int(os.environ.get("MOE_WARMUP_N", "87"))
BRIDGE0 = int(os.environ.get("MOE_BRIDGE0", "18"))
BRIDGE1 = int(os.environ.get("MOE_BRIDGE1", "8"))
WARMUP_COLS = 64


# ----------------------------------------------------------------------------
# Planning: decompose ragged groups into eight 384-pieces + eight 128-pieces.
# ----------------------------------------------------------------------------

def _plan_pieces(group_sizes):
    """Return (pieces384, pieces128): lists of (expert, tok_start), 8 each.

    Requires all group sizes to be multiples of 128 and sum to 4096.
    """
    g = [int(v) for v in group_sizes]
    if sum(g) != NCORES * TOK or any(v % 128 for v in g) or len(g) != E:
        return None
    offs = np.concatenate([[0], np.cumsum(g)])
    x = [v // 384 for v in g]
    total = sum(x)
    if total < 8:
        return None
    e_i = 0
    while total > 8:  # shed surplus 384-pieces (each becomes three 128s)
        if x[e_i] > 0:
            x[e_i] -= 1
            total -= 1
        e_i = (e_i + 1) % E
    p384, p128 = [], []
    for e in range(E):
        t = int(offs[e])
        for _ in range(x[e]):
            p384.append((e, t))
            t += 384
        while t < offs[e + 1]:
            p128.append((e, t))
            t += 128
    if len(p384) != 8 or len(p128) != 8:
        return None
    return p384, p128


# ----------------------------------------------------------------------------
# Device program
# ----------------------------------------------------------------------------

def _build_program(nrs):
    """nrs[s] = token count of chunk/slot s (chunk s uses weight slot s)."""
    nc = bacc.Bacc()
    nchunks = len(nrs)

    xT0a = nc.declare_dram_parameter("xT_0a", [128, KT // 2, nrs[0]], FP16,
                                     isOutput=False)
    xT0b = nc.declare_dram_parameter("xT_0b", [128, KT // 2, nrs[0]], FP16,
                                     isOutput=False)
    xT1 = nc.declare_dram_parameter("xT_1", [128, KT, nrs[1]], FP16,
                                    isOutput=False)
    w13p_d = [
        nc.declare_dram_parameter(f"w13_{s}", [IT, 128, 2 * KT * 128], FP16,
                                  isOutput=False)
        for s in range(nchunks)
    ]
    w2p_d = [
        nc.declare_dram_parameter(f"w2_{s}", [IT, 128, MT * 128], FP16,
                                  isOutput=False)
        for s in range(nchunks)
    ]
    outs = [
        nc.declare_dram_parameter(f"outT_{j}", [MT, 128, nr], F32, isOutput=True)
        for j, nr in enumerate(nrs)
    ]

    with tile.TileContext(nc) as tc:
        with tc.tile_pool(name="xp", bufs=1) as xp, \
             tc.tile_pool(name="w13", bufs=10) as w13p, \
             tc.tile_pool(name="w2", bufs=2 * IT + 1) as w2p, \
             tc.tile_pool(name="h", bufs=2 * IT + 2) as hp, \
             tc.tile_pool(name="su", bufs=4) as sup, \
             tc.tile_pool(name="oc", bufs=6) as ocp, \
             tc.tile_pool(name="acc", bufs=MT) as accp, \
             tc.tile_pool(name="w13h", bufs=1) as hw13p, \
             tc.tile_pool(name="up", bufs=3, space="PSUM") as up, \
             tc.tile_pool(name="vp", bufs=3, space="PSUM") as vp, \
             tc.tile_pool(name="op", bufs=2, space="PSUM") as op:

            # -- warmup train: PE continuously busy until first weights land
            #    (memset on gpsimd: Pool is idle at t=0, shortest dep chain)
            wz = sup.tile([1, 16 + WARMUP_COLS], FP16, tag="warm")
            nc.gpsimd.memset(wz[:], 0.0)
            warm = wz[:][:, 0:16]
            warm_mov = wz[:][:, 16:16 + WARMUP_COLS]
            wps = op.tile([16, WARMUP_COLS], F32, tag="o", name="warmps")
            for _ in range(WARMUP_N):
                nc.tensor.matmul(wps[:], warm, warm_mov, start=True, stop=True)

            # -- head loads, interleaved across SP and ACT queues so the
            #    first matmul's dependencies (w1 of i0 + first half of x)
            #    arrive after only ~2.5us of bus time
            xt0a = xp.tile([128, KT // 2, nrs[0]], FP16, name="xt0a", tag="xt0a")
            xt0b = xp.tile([128, KT // 2, nrs[0]], FP16, name="xt0b", tag="xt0b")
            xt1 = xp.tile([128, KT, nrs[1]], FP16, name="xt1", tag="xt1")
            w1t0 = hw13p.tile([128, KT * 128], FP16, tag="w1h")
            w3t0 = hw13p.tile([128, KT * 128], FP16, tag="w3h")
            nc.sync.dma_start(out=w1t0[:], in_=w13p_d[0][0][:, 0:KT * 128])
            nc.scalar.dma_start(out=xt0a[:], in_=xT0a[:])
            nc.sync.dma_start(out=w3t0[:], in_=w13p_d[0][0][:, KT * 128:])
            nc.scalar.dma_start(out=xt0b[:], in_=xT0b[:])
            nc.scalar.dma_start(out=xt1[:], in_=xT1[:])

            w2ts = {ci: [None] * IT for ci in range(nchunks)}
            hts = {ci: [] for ci in range(nchunks)}

            def load_w2(ci, i):
                t = w2p.tile([128, MT * 128], FP16, tag="w2")
                nc.sync.dma_start(out=t[:], in_=w2p_d[ci][i])
                w2ts[ci][i] = t

            def xk(ci, k):
                if ci == 1:
                    return xt1[:, k, :]
                if k < KT // 2:
                    return xt0a[:, k, :]
                return xt0b[:, k - KT // 2, :]

            def bridge(n):
                for _ in range(n):
                    nc.tensor.matmul(wps[:], warm, warm_mov, start=True, stop=True)

            def stage1_finish(ci, u, v):
                nr = nrs[ci]
                su = sup.tile([128, nr], F32, tag="su")
                nc.scalar.activation(
                    out=su[:], in_=u[:],
                    func=mybir.ActivationFunctionType.Silu,
                )
                ht = hp.tile([128, nr], FP16, tag="h")
                nc.vector.tensor_mul(ht[:], su[:], v[:])
                hts[ci].append(ht)

            def stage1_tile(ci, i):
                nr = nrs[ci]
                w13t = w13p.tile([128, 2 * KT * 128], FP16, tag="w13")
                nc.sync.dma_start(out=w13t[:], in_=w13p_d[ci][i])
                wr = w13t[:].rearrange("p (t k j) -> p t k j", t=2, k=KT)
                u = up.tile([128, nr], F32, tag="u")
                v = vp.tile([128, nr], F32, tag="v")
                for k in range(KT):
                    nc.tensor.matmul(
                        u[:], wr[:, 0, k, :], xk(ci, k),
                        start=(k == 0), stop=(k == KT - 1),
                    )
                for k in range(KT):
                    nc.tensor.matmul(
                        v[:], wr[:, 1, k, :], xk(ci, k),
                        start=(k == 0), stop=(k == KT - 1),
                    )
                stage1_finish(ci, u, v)

            def stage1_tile0():
                # i=0 of chunk 0 with split weight/x halves: consume the a-
                # halves of u and v while the b-half data is still in flight
                KH = KT // 2
                u = up.tile([128, nrs[0]], F32, tag="u")
                v = vp.tile([128, nrs[0]], F32, tag="v")
                w1r = w1t0[:].rearrange("p (k j) -> p k j", k=KT)
                w3r = w3t0[:].rearrange("p (k j) -> p k j", k=KT)
                for k in range(KH):
                    nc.tensor.matmul(u[:], w1r[:, k, :], xk(0, k),
                                     start=(k == 0), stop=False)
                for k in range(KH):
                    nc.tensor.matmul(v[:], w3r[:, k, :], xk(0, k),
                                     start=(k == 0), stop=False)
                bridge(BRIDGE0)
                for k in range(KH, KT):
                    nc.tensor.matmul(u[:], w1r[:, k, :], xk(0, k),
                                     start=False, stop=(k == KT - 1))
                for k in range(KH, KT):
                    nc.tensor.matmul(v[:], w3r[:, k, :], xk(0, k),
                                     start=False, stop=(k == KT - 1))
                stage1_finish(0, u, v)

            # -- phase 1: chunk 0 gets a 2-tile head start (bus ramps while
            #    warmup covers PE), then chunk-paired per i-tile; the first
            #    half of chunk 0's w2 streams into the per-pair bus slack
            IH = IT // 2  # phase-2 half-chain split point for chunk 0
            stage1_tile0()
            stage1_tile(0, 1)
            bridge(BRIDGE1)
            for i in range(IT):
                stage1_tile(1, i)
                if i + 2 < IT:
                    stage1_tile(0, i + 2)
                if 2 <= i < 2 + IH:
                    load_w2(0, i - 2)

            # -- phase 2. Chunk 0 runs as two half-chains over i so only the
            #    first IH w2 tiles must precede it; the rest (and chunk 1's
            #    w2) stream during pass A / phase 2(0).
            for i in range(IH, IT):
                load_w2(0, i)
            for i in range(IT):
                load_w2(1, i)

            accs = []
            for m in range(MT):                       # pass A: i in [0, IH)
                o = op.tile([128, nrs[0]], F32, tag="o")
                for i in range(IH):
                    w2r = w2ts[0][i][:].rearrange("p (m j) -> p m j", m=MT)
                    nc.tensor.matmul(
                        o[:], w2r[:, m, :], hts[0][i][:],
                        start=(i == 0), stop=(i == IH - 1),
                    )
                acc = accp.tile([128, nrs[0]], F32, tag="acc")
                nc.vector.tensor_copy(out=acc[:], in_=o[:])
                accs.append(acc)
            for m in range(MT):                       # pass B: i in [IH, IT)
                o = op.tile([128, nrs[0]], F32, tag="o")
                for i in range(IH, IT):
                    w2r = w2ts[0][i][:].rearrange("p (m j) -> p m j", m=MT)
                    nc.tensor.matmul(
                        o[:], w2r[:, m, :], hts[0][i][:],
                        start=(i == IH), stop=(i == IT - 1),
                    )
                oc = ocp.tile([128, nrs[0]], F32, tag="oc")
                nc.vector.tensor_add(oc[:], accs[m][:], o[:])
                nc.sync.dma_start(out=outs[0][m], in_=oc[:])

            if nchunks > 1:                           # chunk 1: full chains
                nr = nrs[1]
                for m in range(MT):
                    o = op.tile([128, nr], F32, tag="o")
                    for i in range(IT):
                        w2r = w2ts[1][i][:].rearrange("p (m j) -> p m j", m=MT)
                        nc.tensor.matmul(
                            o[:], w2r[:, m, :], hts[1][i][:],
                            start=(i == 0), stop=(i == IT - 1),
                        )
                    oc = ocp.tile([128, nr], F32, tag="oc")
                    nc.vector.tensor_copy(out=oc[:], in_=o[:])
                    nc.sync.dma_start(out=outs[1][m], in_=oc[:])

    nc.finalize()
    return nc


# ----------------------------------------------------------------------------
# Host-side data prep
# ----------------------------------------------------------------------------

def _fmt_w13(w1e, w3e):
    # [H, I] x2 -> [IT, 128, 2*KT*128]; block i holds w1 then w3 k-tiles
    def f(w):
        return w.reshape(KT, 128, IT, 128).transpose(2, 1, 0, 3).reshape(
            IT, 128, KT * 128)
    return np.ascontiguousarray(
        np.concatenate([f(w1e), f(w3e)], axis=2)).astype(np.float16)


def _fmt_w2(w):
    # [I, H] -> [IT, 128, MT*128] (already contiguous blocks)
    return np.ascontiguousarray(w.reshape(IT, 128, MT * 128)).astype(np.float16)


def _fmt_xT(x):
    # [n, H] -> [128, KT, n]; partition p, k, t = x[t, k*128+p]
    n = x.shape[0]
    return np.ascontiguousarray(
        x.T.reshape(KT, 128, n).transpose(1, 0, 2)).astype(np.float16)


_CACHE = {}


def _get_runner(variant, gs_key, group_sizes):
    key = (variant, gs_key)
    if key in _CACHE:
        return _CACHE[key]
    if variant != "fp16":
        raise NotImplementedError(f"variant {variant}")

    plan = _plan_pieces(group_sizes)
    if plan is None:
        raise NotImplementedError(
            f"group_sizes {list(group_sizes)} not decomposable into 384/128 pieces"
        )
    p384, p128 = plan

    nc = _build_program([384, 128])
    runner = _make_pjrt_runner(nc)
    st = {"nc": nc, "runner": runner, "p384": p384, "p128": p128}
    _CACHE[key] = st
    return st


def _make_pjrt_runner(nc):
    """Persistent jit'd SPMD executor (mirrors bass2jax.run_bass_via_pjrt)."""
    from jax.sharding import Mesh, PartitionSpec
    from jax.experimental.shard_map import shard_map
    from concourse.bass2jax import (
        _bass_exec_p, install_neuronx_cc_hook, partition_id_tensor,
    )

    install_neuronx_cc_hook()

    partition_name = nc.partition_id_tensor.name if nc.partition_id_tensor else None
    in_names, out_names, out_avals = [], [], []
    for alloc in nc.m.functions[0].allocations:
        if not isinstance(alloc, mybir.MemoryLocationSet):
            continue
        name = alloc.memorylocations[0].name
        if alloc.kind == "ExternalInput":
            if name != partition_name:
                in_names.append(name)
        elif alloc.kind == "ExternalOutput":
            out_names.append(name)
            out_avals.append(
                jax.core.ShapedArray(tuple(alloc.tensor_shape), mybir.dt.np(alloc.dtype))
            )
    n_params = len(in_names)
    n_outs = len(out_names)
    all_in_names = list(in_names) + list(out_names)
    if partition_name is not None:
        all_in_names.append(partition_name)
    donate = tuple(range(n_params, n_params + n_outs))

    def _body(*args):
        operands = list(args)
        if partition_name is not None:
            operands.append(partition_id_tensor())
        outs = _bass_exec_p.bind(
            *operands,
            out_avals=tuple(out_avals),
            in_names=tuple(all_in_names),
            out_names=tuple(out_names),
            lowering_input_output_aliases=(),
            sim_require_finite=True,
            sim_require_nnan=True,
            nc=nc,
        )
        return tuple(outs)

    devices = jax.devices()[:NCORES]
    mesh = Mesh(np.asarray(devices), ("core",))
    in_specs = (PartitionSpec("core"),) * (n_params + n_outs)
    out_specs = (PartitionSpec("core"),) * n_outs
    jitted = jax.jit(
        shard_map(_body, mesh=mesh, in_specs=in_specs, out_specs=out_specs,
                  check_rep=False),
        donate_argnums=donate, keep_unused=True,
    )

    def run(in_maps):
        per_core = [[np.asarray(m[n]) for n in in_names] for m in in_maps]
        concat_in = [
            np.concatenate([per_core[c][i] for c in range(NCORES)], axis=0)
            for i in range(n_params)
        ]
        zeros = [
            np.zeros((NCORES * a.shape[0], *a.shape[1:]), a.dtype) for a in out_avals
        ]
        out_arrs = jitted(*concat_in, *zeros)
        return [
            {
                name: np.asarray(out_arrs[i]).reshape(NCORES, *out_avals[i].shape)[c]
                for i, name in enumerate(out_names)
            }
            for c in range(NCORES)
        ]

    return run


def _prep_in_maps(st, hidden_states, w1, w2, w3):
    w13f = [_fmt_w13(np.asarray(w1[e]), np.asarray(w3[e])) for e in range(E)]
    w2f = [_fmt_w2(np.asarray(w2[e])) for e in range(E)]
    hs = np.asarray(hidden_states)

    in_maps = []
    for c in range(NCORES):
        eA, tA = st["p384"][c]
        eB, tB = st["p128"][c]
        xA = _fmt_xT(hs[tA:tA + 384])
        in_maps.append({
            "xT_0a": np.ascontiguousarray(xA[:, 0:KT // 2, :]),
            "xT_0b": np.ascontiguousarray(xA[:, KT // 2:, :]),
            "xT_1": _fmt_xT(hs[tB:tB + 128]),
            "w13_0": w13f[eA], "w2_0": w2f[eA],
            "w13_1": w13f[eB], "w2_1": w2f[eB],
        })
    return in_maps


def _assemble(st, results, out_dtype):
    out = np.empty((NCORES * TOK, H), dtype=out_dtype)
    for c in range(NCORES):
        eA, tA = st["p384"][c]
        eB, tB = st["p128"][c]
        oA = results[c]["outT_0"].reshape(H, 384)   # [MT,128,384] -> [H,384]
        oB = results[c]["outT_1"].reshape(H, 128)
        out[tA:tA + 384] = oA.T
        out[tB:tB + 128] = oB.T
    return out


def kernel(hidden_states, group_sizes, w1, w2, w3):
    gs = np.asarray(group_sizes)
    st = _get_runner(VARIANT, gs.tobytes(), gs)
    in_maps = _prep_in_maps(st, hidden_states, w1, w2, w3)
    results = st["runner"](in_maps)
    return _assemble(st, results, np.asarray(hidden_states).dtype)
